# revision 12
# baseline (speedup 1.0000x reference)
"""Trainium2 Bass kernel for nn_BasisDense: y = einsum('bd,duk,bk->bu', x, kernel, c_prob) + bias.

Strategy:
  - Factorize: t[b,(u,k)] = x @ kernel2d  (kernel2d = kernel.reshape(D, U*K)), then
    y[b,u] = sum_k t[b,u,k]*c_prob[b,k] + bias[u] (DVE epilogue).
  - Hybrid shard across 8 cores: batch B into 4 x units U into 2.
  - Mixed precision: d-tiles 0..13 run bf16 (1 cyc/row); d-tiles 14..15 run as ONE
    fp8e4m3 DoubleRow matmul (2 MACs/cell/cycle) into a second PSUM bank, scaled
    by 64 on the kernel side (fp8 subnormal floor) and folded back in the epilogue
    via a 1/64-scaled c_prob replica. Total rms rel err ~1.35e-2 (tolerance 2e-2).
  - Kernel-chunk DMAs are 1024 cols wide (2KB per partition line) -> ~2x the
    per-queue DMA packet rate (queues are ~10ns/packet bound).
  - First output tile runs t-outer/bt-inner across all 8 PSUM banks so the
    demand for x/kernel chunks ramps gradually instead of spiking at MM #0.
  - Host-side input marshaling: transposes/casts/packing, O(B*D + D*U/64) work.
"""
import sys

sys.path.insert(0, "/opt/trn_rl_repo")

import numpy as np
import concourse.bacc as bacc
import concourse.mybir as mybir
import concourse.tile as tile
from concourse import bass_utils

B, D, U, K = 4096, 2048, 2048, 8
NCORES = 8
SHARD_U = 2  # units-dimension shards
SHARD_B = NCORES // SHARD_U
BS = B // SHARD_B  # batch rows per core
USH = U // SHARD_U  # units per core
UKS = USH * K  # fused (u,k) output columns per core
NFREE = 512  # matmul moving free dim (1 PSUM bank of fp32)
NW = 1024  # kernel-chunk width (2 n-slices -> 2KB DMA packets)
NG = UKS // NW  # kernel chunk groups
NT = UKS // NFREE  # n-tiles
DT = D // 128  # contraction tiles
DTB = DT - 2  # bf16 contraction tiles (last 2 ride the fp8 DoubleRow matmul)
BT = BS // 128  # batch partition-tiles per core
UPT = NFREE // K  # u-columns produced per n-tile
KT_BUFS = 3
K8SCALE = 64.0  # fp8 kernel pre-scale (keeps values out of the subnormal mud)

_CACHE = {}


def _build():
    nc = bacc.Bacc("TRN2", target_bir_lowering=False, debug=False, num_devices=NCORES)
    f32 = mybir.dt.float32
    bf16 = mybir.dt.bfloat16
    f8 = mybir.dt.float8e4

    xt = nc.dram_tensor("xt", [D, BS], bf16, kind="ExternalInput").ap()
    x8 = nc.dram_tensor("x8", [128, 2, BS], f8, kind="ExternalInput").ap()
    cp = nc.dram_tensor("cp", [128, BT * K], f32, kind="ExternalInput").ap()
    cp64 = nc.dram_tensor("cp64", [128, BT * K], f32, kind="ExternalInput").ap()
    kern = nc.dram_tensor("kern", [D, USH, K], bf16, kind="ExternalInput").ap()
    kern8 = nc.dram_tensor("kern8", [128, 2, UKS], f8, kind="ExternalInput").ap()
    biasr = nc.dram_tensor("biasr", [128, USH], f32, kind="ExternalInput").ap()
    y = nc.dram_tensor("y", [BS, USH], f32, kind="ExternalOutput").ap()

    # [128 d-partition, DT, UKS] view of this core's kernel2d shard
    kern2d = kern.rearrange("(t p) u k -> p t (u k)", p=128)

    with tile.TileContext(nc) as tc:
        with (
            tc.tile_pool(name="const", bufs=1) as constp,
            tc.tile_pool(name="kt", bufs=KT_BUFS) as ktp,
            tc.tile_pool(name="k8p", bufs=KT_BUFS) as k8p,
            tc.tile_pool(name="mps", bufs=8, space="PSUM") as mps,
            tc.tile_pool(name="ep", bufs=6) as epp,
            tc.tile_pool(name="yp", bufs=16) as ypp,
        ):
            xT = constp.tile([128, DT, BS], bf16)  # [d-part, d-tile, b]
            x8T = constp.tile([128, 2, BS], f8)
            c_rep = constp.tile([128, BT, NFREE], f32)
            c_rep64 = constp.tile([128, BT, NFREE], f32)
            bias_rep = constp.tile([128, USH], f32)

            # xT/x8T ride the gpsimd (SWDGE) queue, off the two HWDGE queues
            # that carry the kernel-chunk stream
            xt_v = xt.rearrange("(t p) b -> p t b", p=128)
            c_nat = constp.tile([128, BT, K], f32)
            c64_nat = constp.tile([128, BT, K], f32)
            nc.scalar.dma_start(c_nat, cp.rearrange("p (bt k) -> p bt k", k=K))
            nc.scalar.dma_start(c64_nat, cp64.rearrange("p (bt k) -> p bt k", k=K))
            for t in range(DT):
                nc.gpsimd.dma_start(xT[:, t, :], xt_v[:, t, :])
            nc.gpsimd.dma_start(x8T, x8)
            # replicate c_prob 64x along the free dim on the DVE (tiny)
            for rep, nat in ((c_rep, c_nat), (c_rep64, c64_nat)):
                for bt in range(BT):
                    nc.vector.tensor_copy(rep[:, bt, 0:K], nat[:, bt, :])
                    s = K
                    while s < NFREE:
                        nc.vector.tensor_copy(rep[:, bt, s : 2 * s], rep[:, bt, 0:s])
                        s *= 2

            # dummy matmuls on a zeroed tile: warms the PE HAM clock-gate to
            # 8/8 during the DMA fill phase, before real data lands
            wz = constp.tile([128, NFREE + 128], bf16)
            nc.vector.memset(wz, 0)
            warm = mps.tile([128, NFREE], f32, tag="acc", name="warm")
            for _ in range(14):
                nc.tensor.matmul(
                    warm, wz[:, NFREE:], wz[:, 0:NFREE], start=True, stop=True
                )

            def epilogue(acc, acc8, bt, n):
                # y[b, u] = sum_k (acc + acc8/64)[b, (u,k)] * c[b, k] + bias[u]
                tmp = epp.tile([128, NFREE], f32, tag="tmp")
                nc.vector.tensor_mul(tmp, acc, c_rep[:, bt, :])
                if acc8 is not None:
                    tmp8 = epp.tile([128, NFREE], f32, tag="tmp8")
                    nc.vector.tensor_mul(tmp8, acc8, c_rep64[:, bt, :])
                    nc.vector.tensor_add(tmp, tmp, tmp8)
                yt = ypp.tile([128, UPT], f32, tag="yt")
                nc.vector.tensor_reduce(
                    yt,
                    tmp.rearrange("p (u k) -> p u k", k=K),
                    axis=mybir.AxisListType.X,
                    op=mybir.AluOpType.add,
                )
                yf = ypp.tile([128, UPT], f32, tag="yf")
                nc.vector.tensor_add(yf, yt, bias_rep[:, n * UPT : (n + 1) * UPT])
                # output DMAs ride the scalar engine's HWDGE queue
                nc.scalar.dma_start(
                    y[bt * 128 : (bt + 1) * 128, n * UPT : (n + 1) * UPT],
                    yf,
                )

            for g in range(NG):
                kt = ktp.tile([128, DT, NW], bf16, tag="kt")
                # per-d-tile chunk DMAs (2KB/partition): the t-th matmul can
                # start as soon as chunk t lands, alternating across queues.
                # g=0 also needs t=14,15 in bf16 for the n=0 first pass.
                tmax = DT if g == 0 else DTB
                for t in range(tmax):
                    eng = nc.sync if t % 2 == 0 else nc.scalar
                    eng.dma_start(
                        kt[:, t, :],
                        kern2d[:, t, g * NW : (g + 1) * NW],
                    )
                k8 = k8p.tile([128, 2, NW], f8, tag="k8")
                nc.gpsimd.dma_start(k8, kern8[:, :, g * NW : (g + 1) * NW])
                if g == 0:
                    nc.scalar.dma_start(bias_rep, biasr)
                def bf16_group(bt, hs, nt):
                    acc = mps.tile([128, NFREE], f32, tag="acc", name="acc")
                    for t in range(nt):
                        nc.tensor.matmul(
                            acc,
                            xT[:, t, bt * 128 : (bt + 1) * 128],
                            kt[:, t, hs : hs + NFREE],
                            start=(t == 0),
                            stop=(t == nt - 1),
                        )
                    return acc

                def dr_group(bt, hs):
                    acc8 = mps.tile([128, NFREE], f32, tag="acc", name="acc8")
                    nc.tensor.matmul(
                        acc8,
                        x8T[:, :, bt * 128 : (bt + 1) * 128],
                        k8[:, :, hs : hs + NFREE],
                        start=True,
                        stop=True,
                        perf_mode=mybir.MatmulPerfMode.DoubleRow,
                    )
                    return acc8

                if g == 0:
                    # first pass: all-bf16, t-outer/bt-inner across all 8
                    # PSUM banks so each chunk is needed only every 8 MMs
                    accs = [
                        mps.tile([128, NFREE], f32, tag="acc", name=f"acc_n0_{bt}")
                        for bt in range(BT)
                    ]
                    for t in range(DT):
                        for bt in range(BT):
                            nc.tensor.matmul(
                                accs[bt],
                                xT[:, t, bt * 128 : (bt + 1) * 128],
                                kt[:, t, 0:NFREE],
                                start=(t == 0),
                                stop=(t == DT - 1),
                            )
                    for bt in range(BT):
                        epilogue(accs[bt], None, bt, 0)
                    for bt in range(BT):
                        acc = bf16_group(bt, NFREE, DTB)
                        acc8 = dr_group(bt, NFREE)
                        epilogue(acc, acc8, bt, 1)
                else:
                    # merged bt-group: both halves' bf16 streams, then both DR
                    # matmuls back-to-back (identical stationary weights)
                    for bt in range(BT):
                        acc_a = bf16_group(bt, 0, DTB)
                        acc_b = bf16_group(bt, NFREE, DTB)
                        acc8a = dr_group(bt, 0)
                        acc8b = dr_group(bt, NFREE)
                        epilogue(acc_a, acc8a, bt, 2 * g)
                        epilogue(acc_b, acc8b, bt, 2 * g + 1)
    nc.compile()
    return nc


def _in_maps(x, c_prob, kernel, bias):
    import ml_dtypes

    bf16 = ml_dtypes.bfloat16
    e4m3 = ml_dtypes.float8_e4m3
    x = np.ascontiguousarray(x, dtype=np.float32)
    c_prob = np.ascontiguousarray(c_prob, dtype=np.float32)
    kernel = np.ascontiguousarray(kernel, dtype=np.float32)
    bias = np.ascontiguousarray(bias, dtype=np.float32)
    d8 = DTB * 128  # first d-row handled by the fp8 DoubleRow pair
    maps = []
    for c in range(NCORES):
        bq, uh = c % SHARD_B, c // SHARD_B
        xs = x[bq * BS : (bq + 1) * BS]
        cs = c_prob[bq * BS : (bq + 1) * BS]
        # pack per-partition: cp[p, bt*K+k] = c_prob[bt*128+p, k]
        cpk = np.ascontiguousarray(
            cs.reshape(BT, 128, K).transpose(1, 0, 2).reshape(128, BT * K)
        )
        ks = kernel[:, uh * USH : (uh + 1) * USH, :]
        k8s = (ks[d8:].reshape(2, 128, UKS) * K8SCALE).astype(e4m3)
        x8s = np.ascontiguousarray(xs[:, d8:].T).reshape(2, 128, BS)
        maps.append(
            {
                "xt": np.ascontiguousarray(xs.T).astype(bf16),
                "x8": np.ascontiguousarray(x8s.transpose(1, 0, 2)).astype(e4m3),
                "cp": cpk,
                "cp64": cpk / np.float32(K8SCALE),
                "kern": np.ascontiguousarray(ks).astype(bf16),
                "kern8": np.ascontiguousarray(k8s.transpose(1, 0, 2)),
                "biasr": np.ascontiguousarray(
                    np.broadcast_to(bias[uh * USH : (uh + 1) * USH], (128, USH))
                ),
            }
        )
    return maps


def kernel(x, c_prob, kernel, bias):
    if "nc" not in _CACHE:
        _CACHE["nc"] = _build()
    nc = _CACHE["nc"]
    res = bass_utils.run_bass_kernel_spmd(
        nc, _in_maps(x, c_prob, kernel, bias), list(range(NCORES))
    )
    out = np.empty((B, U), dtype=np.float32)
    for c in range(NCORES):
        bq, uh = c % SHARD_B, c // SHARD_B
        out[bq * BS : (bq + 1) * BS, uh * USH : (uh + 1) * USH] = res.results[c]["y"]
    return out
